# revision 7
# baseline (speedup 1.0000x reference)
"""Trainium2 Bass kernel for IntersectionalVolumeRatio.

out[m,n] = exp(sum_d log(softplus(min(Zm,Ze) - max(zm,ze))) - lmv[m])
with men boxes [M=256, D=64] and candidate boxes [N=20000, D=64],
sharded over 8 NeuronCores along the candidate axis (2500 each).

Device math (exp commutes with min/max; keeps all activations in the
single `natural_log_exp` ACT table set):
  u    = e^diff = min(e^Zm, e^Ze) * min(e^-zm, e^-ze)
  sp   = ln(1+u) = softplus(diff)          [ACT pass 1]
  lspq = ln(sp) -> f16                     [ACT pass 2]
  PE sums lspq over d via a sliding 0/1 f16 window into PSUM
  out  = exp(psum - lmv[m]) -> bf16        [per-partition bias epilogue]

Per-core layout: partitions = 128 = [d(64) of mention 2j | d(64) of
mention 2j+1], free axis = candidate shard (2500). 128 iterations of
(tensor_scalar min, scalar_tensor_tensor min*mult, 2 ACT passes,
5 PSUM matmuls) cover all 256 mentions. The kernel is ACT-bound:
2 LUT passes/element is the algorithmic floor with the available
activation-function table sets.

Host wrapper: the Bass module and the jitted shard_map executable are
built once and cached; input arrays are cached device-side and
re-uploaded only when their content changes; mention log-volumes are
computed on host (tiny) and folded into the epilogue bias. Output
returns as bf16 (halves the device->host transfer) and is upcast to
f32 on host.

_build(reps=R) unrolls the compute loop R times inside the NEFF
(outputs unchanged — PSUM restarts every rep); timing two dispatch
variants (R=17 vs R=33) and taking the slope isolates pure on-device
execution time from dispatch/transfer overhead. Used by test.py.
"""

import numpy as np

M = 256
D = 64
N = 20000
NCORES = 8
NS = N // NCORES          # 2500 candidates per core
CH = 500                  # PSUM chunk (PSUM bank limit is 512 f32)
NCH = NS // CH

_cache = {}


def _build(reps=1):
    from concourse import bacc, mybir
    from concourse.tile import TileContext

    F32 = mybir.dt.float32
    F16 = mybir.dt.float16
    BF16 = mybir.dt.bfloat16
    AF = mybir.ActivationFunctionType
    OP = mybir.AluOpType

    nc = bacc.Bacc("TRN2", target_bir_lowering=False, debug=False,
                   num_devices=NCORES)
    zet = nc.dram_tensor("zet", [64, NS], F32, kind="ExternalInput").ap()
    nzet = nc.dram_tensor("nzet", [64, NS], F32, kind="ExternalInput").ap()
    ezm = nc.dram_tensor("ezm", [128, 128], F32, kind="ExternalInput").ap()
    enzm = nc.dram_tensor("enzm", [128, 128], F32, kind="ExternalInput").ap()
    nlmv = nc.dram_tensor("nlmv", [128, 2], F32, kind="ExternalInput").ap()
    out = nc.dram_tensor("out", [M, NS], BF16, kind="ExternalOutput").ap()

    KB = 4                # mention-pairs fused per ACT instruction
    with TileContext(nc) as tc:
        with tc.tile_pool(name="persist", bufs=1) as pp, \
             tc.tile_pool(name="ubig", bufs=2) as up, \
             tc.tile_pool(name="lsp", bufs=4) as lp, \
             tc.tile_pool(name="work", bufs=2) as wp, \
             tc.tile_pool(name="psum", bufs=1, space="PSUM") as qp:

            # ---- stage inputs ----
            zet_sb = pp.tile([64, NS], F32, tag="zet")
            nzet_sb = pp.tile([64, NS], F32, tag="nzet")
            ezm_sb = pp.tile([128, 128], F32, tag="ezm")
            enzm_sb = pp.tile([128, 128], F32, tag="enzm")
            nlmv_sb = pp.tile([128, 2], F32, tag="nlmv")
            for t_, s_ in [(zet_sb, zet), (nzet_sb, nzet), (ezm_sb, ezm),
                           (enzm_sb, enzm), (nlmv_sb, nlmv)]:
                nc.sync.dma_start(out=t_[:], in_=s_[:])

            # ---- exp of candidates, duplicated into both partition halves
            EZe = pp.tile([128, NS], F32, tag="EZe")
            Enze = pp.tile([128, NS], F32, tag="Enze")
            nc.scalar.activation(EZe[0:64, :], zet_sb[:], AF.Exp)
            nc.scalar.activation(EZe[64:128, :], zet_sb[:], AF.Exp)
            nc.scalar.activation(Enze[0:64, :], nzet_sb[:], AF.Exp)
            nc.scalar.activation(Enze[64:128, :], nzet_sb[:], AF.Exp)

            # ---- sliding ones window for the d-reduction (fp16) ----
            G = pp.tile([128, 192], F16, tag="G")
            nc.vector.memset(G[:], 0.0)
            nc.vector.memset(G[0:64, 64:65], 1.0)
            nc.vector.memset(G[64:128, 128:129], 1.0)

            # ---- main loop (reps>1 only for HW-timing variants) ----
            # KB mention-pairs share one [128, KB*NS] tile: the two min
            # passes write each NS-slice in place (TS then in-place STT),
            # then a single Ln1p (in place) + single Ln->f16 cover all KB
            # pairs, amortizing ACT fixed costs and semaphore traffic.
            for _rep in range(reps):
                for g in range(2):
                    psums = [qp.tile([128, CH], F32, name=f"ps{c}",
                                     tag=f"ps{c}") for c in range(NCH)]
                    for t in range(64 // KB):
                        uK = up.tile([128, KB * NS], F32, tag="uK")
                        for kk in range(KB):
                            mp = 64 * g + KB * t + kk
                            us = uK[:, kk * NS:(kk + 1) * NS]
                            nc.vector.tensor_scalar(
                                us, Enze[:], enzm_sb[:, mp:mp + 1], None,
                                OP.min)
                            nc.vector.scalar_tensor_tensor(
                                us, EZe[:], ezm_sb[:, mp:mp + 1], us,
                                OP.min, OP.mult)
                        nc.scalar.activation(uK[:], uK[:], AF.Ln, bias=1.0)
                        lspqK = lp.tile([128, KB * NS], F16, tag="lspqK")
                        nc.scalar.activation(lspqK[:], uK[:], AF.Ln)
                        for kk in range(KB):
                            j = KB * t + kk
                            for c in range(NCH):
                                cs = slice(kk * NS + c * CH,
                                           kk * NS + (c + 1) * CH)
                                nc.tensor.matmul(
                                    psums[c][:], lhsT=G[:, 64 - j:192 - j],
                                    rhs=lspqK[:, cs], start=(j == 0),
                                    stop=(j == 63))
                    for c in range(NCH):
                        cs = slice(c * CH, (c + 1) * CH)
                        osb = wp.tile([128, CH], BF16, tag="osb")
                        nc.scalar.activation(osb[:], psums[c][:], AF.Exp,
                                             bias=nlmv_sb[:, g:g + 1])
                        nc.sync.dma_start(
                            out=out[g * 128:(g + 1) * 128, cs], in_=osb[:])
    nc.compile()
    return nc


def _row_perm():
    # psum partition p in group g holds mention 2*(64g+p) (p<64) or
    # 2*(64g+p-64)+1 (p>=64)
    perm = np.zeros(M, dtype=np.int64)
    for g in range(2):
        for p in range(128):
            men = 2 * (64 * g + p) if p < 64 else 2 * (64 * g + p - 64) + 1
            perm[g * 128 + p] = men
    return perm


def _prep_host(men_embeds, all_en_embeds):
    men = np.ascontiguousarray(np.asarray(men_embeds, dtype=np.float32))
    en = np.ascontiguousarray(np.asarray(all_en_embeds, dtype=np.float32))
    zm, Zm = men[:, :D], men[:, D:]
    # mention-pair column layout: col mp -> mentions (2mp, 2mp+1)
    ezm = np.exp(np.concatenate([Zm[0::2].T, Zm[1::2].T], axis=0))
    enzm = np.exp(-np.concatenate([zm[0::2].T, zm[1::2].T], axis=0))
    ezm = np.ascontiguousarray(ezm, dtype=np.float32)
    enzm = np.ascontiguousarray(enzm, dtype=np.float32)
    # -log mention volume in psum-row layout: nlmv[p, g]
    lmv = np.sum(np.log(np.logaddexp(0.0, Zm - zm)), axis=1)  # [M]
    perm = _row_perm()
    nlmv = np.empty((128, 2), dtype=np.float32)
    for g in range(2):
        nlmv[:, g] = -lmv[perm[g * 128:(g + 1) * 128]]
    zet_all = np.empty((NCORES * 64, NS), dtype=np.float32)
    nzet_all = np.empty((NCORES * 64, NS), dtype=np.float32)
    for s in range(NCORES):
        ens = en[s * NS:(s + 1) * NS]
        zet_all[s * 64:(s + 1) * 64] = ens[:, D:].T      # Ze.T
        nzet_all[s * 64:(s + 1) * 64] = -ens[:, :D].T    # -ze.T
    return {"zet": zet_all, "nzet": nzet_all,
            "ezm": np.tile(ezm, (NCORES, 1)),
            "enzm": np.tile(enzm, (NCORES, 1)),
            "nlmv": np.tile(nlmv, (NCORES, 1))}


def _make_executable(nc):
    """Jit a shard_map over 8 cores around the single bass_exec call."""
    import jax
    from jax.sharding import Mesh, PartitionSpec
    from jax.experimental.shard_map import shard_map
    from concourse import mybir
    from concourse.bass2jax import (
        install_neuronx_cc_hook, _bass_exec_p, partition_id_tensor)

    install_neuronx_cc_hook()
    partition_name = (nc.partition_id_tensor.name
                      if nc.partition_id_tensor else None)
    in_names, out_names, out_avals = [], [], []
    for alloc in nc.m.functions[0].allocations:
        if not isinstance(alloc, mybir.MemoryLocationSet):
            continue
        name = alloc.memorylocations[0].name
        if alloc.kind == "ExternalInput":
            if name != partition_name:
                in_names.append(name)
        elif alloc.kind == "ExternalOutput":
            out_names.append(name)
            out_avals.append(jax.core.ShapedArray(
                tuple(alloc.tensor_shape), mybir.dt.np(alloc.dtype)))
    all_in_names = list(in_names)
    if partition_name is not None:
        all_in_names.append(partition_name)

    def _body(*args):
        operands = list(args)
        if partition_name is not None:
            operands.append(partition_id_tensor())
        return tuple(_bass_exec_p.bind(
            *operands,
            out_avals=tuple(out_avals),
            in_names=tuple(all_in_names),
            out_names=tuple(out_names),
            lowering_input_output_aliases=(),
            sim_require_finite=True,
            sim_require_nnan=True,
            nc=nc,
        ))

    devices = jax.devices()[:NCORES]
    mesh = Mesh(np.asarray(devices), ("core",))
    fn = jax.jit(shard_map(
        _body, mesh=mesh,
        in_specs=(PartitionSpec("core"),) * len(in_names),
        out_specs=(PartitionSpec("core"),) * len(out_names),
        check_rep=False))
    return fn, in_names, mesh


def _get_state():
    if "fn" not in _cache:
        nc = _build()
        fn, in_names, mesh = _make_executable(nc)
        _cache.update(nc=nc, fn=fn, in_names=in_names, mesh=mesh,
                      perm=_row_perm(), host_in={}, dev_in={})
    return _cache


def _device_inputs(st, host_in):
    """Sharded device_put per input array, reusing cached device arrays
    for arrays whose content is unchanged since the previous call."""
    import jax
    from jax.sharding import NamedSharding, PartitionSpec
    sh = NamedSharding(st["mesh"], PartitionSpec("core"))
    dev = []
    for k in st["in_names"]:
        cached = st["host_in"].get(k)
        if cached is None or not np.array_equal(cached, host_in[k]):
            st["dev_in"][k] = jax.device_put(host_in[k], sh)
            st["host_in"][k] = host_in[k]
        dev.append(st["dev_in"][k])
    return dev


def kernel(men_embeds, all_en_embeds):
    st = _get_state()
    dev = _device_inputs(st, _prep_host(men_embeds, all_en_embeds))
    out_arrs = st["fn"](*dev)
    glob = np.asarray(out_arrs[0])                  # [8*256, 2500] bf16
    perm = st["perm"]
    out = np.empty((M, N), dtype=np.float32)
    blocks = glob.astype(np.float32).reshape(NCORES, M, NS)
    for s in range(NCORES):
        out[perm, s * NS:(s + 1) * NS] = blocks[s]
    return out


def hw_exec_time_ns(men_embeds, all_en_embeds, r_lo=17, r_hi=33, ntrials=13):
    """Per-execution on-device time, measured as the wall-time slope
    between NEFFs running the compute loop r_lo vs r_hi times internally
    (identical dispatch/transfer overhead cancels out)."""
    import time
    import jax
    st = _get_state()
    dev = _device_inputs(st, _prep_host(men_embeds, all_en_embeds))
    key = ("fns_timing", r_lo, r_hi)
    if key not in st:
        fns = {}
        for r in (r_lo, r_hi):
            fn_r, _, _ = _make_executable(_build(reps=r))
            fns[r] = fn_r
        st[key] = fns
    fns = st[key]
    for fn in fns.values():                        # warm/compile
        jax.block_until_ready(fn(*dev))
    walls = {r: [] for r in fns}
    for _ in range(ntrials):
        for r in (r_lo, r_hi):                     # adjacent in time so
            t0 = time.time()                       # dispatch conditions pair
            jax.block_until_ready(fns[r](*dev))
            walls[r].append(time.time() - t0)
    diffs = sorted(h - l for l, h in zip(walls[r_lo], walls[r_hi]))
    med_diff = diffs[len(diffs) // 2]
    return med_diff / (r_hi - r_lo) * 1e9, walls


# revision 8
# speedup vs baseline: 1.2138x; 1.2138x over previous
"""Trainium2 Bass kernel for IntersectionalVolumeRatio.

out[m,n] = exp(sum_d log(softplus(min(Zm,Ze) - max(zm,ze))) - lmv[m])
with men boxes [M=256, D=64] and candidate boxes [N=20000, D=64],
sharded over 8 NeuronCores along the candidate axis (2500 each).

Device math (exp commutes with min/max; keeps all activations in the
single `natural_log_exp` ACT table set):
  u    = e^diff = min(e^Zm, e^Ze) * min(e^-zm, e^-ze)
  sp   = ln(1+u) = softplus(diff)          [ACT pass 1]
  lspq = ln(sp) -> f16                     [ACT pass 2]
  PE sums lspq over d via a sliding 0/1 f16 window into PSUM
  out  = exp(psum - lmv[m]) -> bf16        [per-partition bias epilogue]

Per-core layout: partitions = 128 = [d(64) of mention 2j | d(64) of
mention 2j+1], free axis = candidate shard (2500). 128 iterations of
(tensor_scalar min, scalar_tensor_tensor min*mult, 2 ACT passes,
5 PSUM matmuls) cover all 256 mentions. The kernel is ACT-bound:
2 LUT passes/element is the algorithmic floor with the available
activation-function table sets.

Host wrapper: the Bass module and the jitted shard_map executable are
built once and cached; input arrays are cached device-side and
re-uploaded only when their content changes; mention log-volumes are
computed on host (tiny) and folded into the epilogue bias. Output
returns as bf16 (halves the device->host transfer) and is upcast to
f32 on host.

_build(reps=R) unrolls the compute loop R times inside the NEFF
(outputs unchanged — PSUM restarts every rep); timing two dispatch
variants (R=17 vs R=33) and taking the slope isolates pure on-device
execution time from dispatch/transfer overhead. Used by test.py.
"""

import numpy as np

M = 256
D = 64
N = 20000
NCORES = 8
NS = N // NCORES          # 2500 candidates per core
CH = 500                  # PSUM chunk (PSUM bank limit is 512 f32)
NCH = NS // CH

_cache = {}


def _build(reps=1):
    from concourse import bacc, mybir
    from concourse.tile import TileContext

    F32 = mybir.dt.float32
    F16 = mybir.dt.float16
    BF16 = mybir.dt.bfloat16
    AF = mybir.ActivationFunctionType
    OP = mybir.AluOpType

    nc = bacc.Bacc("TRN2", target_bir_lowering=False, debug=False,
                   num_devices=NCORES)
    zet = nc.dram_tensor("zet", [64, NS], F32, kind="ExternalInput").ap()
    nzet = nc.dram_tensor("nzet", [64, NS], F32, kind="ExternalInput").ap()
    ezm = nc.dram_tensor("ezm", [128, 128], F32, kind="ExternalInput").ap()
    enzm = nc.dram_tensor("enzm", [128, 128], F32, kind="ExternalInput").ap()
    nlmv = nc.dram_tensor("nlmv", [128, 2], F32, kind="ExternalInput").ap()
    out = nc.dram_tensor("out", [M, NS], BF16, kind="ExternalOutput").ap()

    KB = 4                # mention-pairs fused per ACT instruction
    with TileContext(nc) as tc:
        with tc.tile_pool(name="persist", bufs=1) as pp, \
             tc.tile_pool(name="ubig", bufs=2) as up, \
             tc.tile_pool(name="lsp", bufs=4) as lp, \
             tc.tile_pool(name="work", bufs=5) as wp, \
             tc.tile_pool(name="psum", bufs=1, space="PSUM") as qp:

            # ---- stage inputs ----
            zet_sb = pp.tile([64, NS], F32, tag="zet")
            nzet_sb = pp.tile([64, NS], F32, tag="nzet")
            ezm_sb = pp.tile([128, 128], F32, tag="ezm")
            enzm_sb = pp.tile([128, 128], F32, tag="enzm")
            nlmv_sb = pp.tile([128, 2], F32, tag="nlmv")
            for t_, s_ in [(zet_sb, zet), (nzet_sb, nzet), (ezm_sb, ezm),
                           (enzm_sb, enzm), (nlmv_sb, nlmv)]:
                nc.sync.dma_start(out=t_[:], in_=s_[:])

            # ---- exp of candidates, duplicated into both partition halves
            EZe = pp.tile([128, NS], F32, tag="EZe")
            Enze = pp.tile([128, NS], F32, tag="Enze")
            nc.scalar.activation(EZe[0:64, :], zet_sb[:], AF.Exp)
            nc.scalar.activation(EZe[64:128, :], zet_sb[:], AF.Exp)
            nc.scalar.activation(Enze[0:64, :], nzet_sb[:], AF.Exp)
            nc.scalar.activation(Enze[64:128, :], nzet_sb[:], AF.Exp)

            # ---- sliding ones window for the d-reduction (fp16) ----
            G = pp.tile([128, 192], F16, tag="G")
            nc.vector.memset(G[:], 0.0)
            nc.vector.memset(G[0:64, 64:65], 1.0)
            nc.vector.memset(G[64:128, 128:129], 1.0)

            # ---- main loop (reps>1 only for HW-timing variants) ----
            # KB mention-pairs share one [128, KB*NS] tile: the two min
            # passes write each NS-slice in place (TS then in-place STT),
            # then a single Ln1p (in place) + single Ln->f16 cover all KB
            # pairs, amortizing ACT fixed costs and semaphore traffic.
            for _rep in range(reps):
                for g in range(2):
                    psums = [qp.tile([128, CH], F32, name=f"ps{c}",
                                     tag=f"ps{c}") for c in range(NCH)]
                    for t in range(64 // KB):
                        uK = up.tile([128, KB * NS], F32, tag="uK")
                        for kk in range(KB):
                            mp = 64 * g + KB * t + kk
                            us = uK[:, kk * NS:(kk + 1) * NS]
                            nc.vector.tensor_scalar(
                                us, Enze[:], enzm_sb[:, mp:mp + 1], None,
                                OP.min)
                            nc.vector.scalar_tensor_tensor(
                                us, EZe[:], ezm_sb[:, mp:mp + 1], us,
                                OP.min, OP.mult)
                        nc.scalar.activation(uK[:], uK[:], AF.Ln, bias=1.0)
                        lspqK = lp.tile([128, KB * NS], F16, tag="lspqK")
                        nc.scalar.activation(lspqK[:], uK[:], AF.Ln)
                        for kk in range(KB):
                            j = KB * t + kk
                            for c in range(NCH):
                                cs = slice(kk * NS + c * CH,
                                           kk * NS + (c + 1) * CH)
                                nc.tensor.matmul(
                                    psums[c][:], lhsT=G[:, 64 - j:192 - j],
                                    rhs=lspqK[:, cs], start=(j == 0),
                                    stop=(j == 63))
                    for c in range(NCH):
                        cs = slice(c * CH, (c + 1) * CH)
                        osb = wp.tile([128, CH], BF16, tag="osb")
                        nc.scalar.activation(osb[:], psums[c][:], AF.Exp,
                                             bias=nlmv_sb[:, g:g + 1])
                        nc.sync.dma_start(
                            out=out[g * 128:(g + 1) * 128, cs], in_=osb[:])
    nc.compile()
    return nc


def _row_perm():
    # psum partition p in group g holds mention 2*(64g+p) (p<64) or
    # 2*(64g+p-64)+1 (p>=64)
    perm = np.zeros(M, dtype=np.int64)
    for g in range(2):
        for p in range(128):
            men = 2 * (64 * g + p) if p < 64 else 2 * (64 * g + p - 64) + 1
            perm[g * 128 + p] = men
    return perm


def _prep_host(men_embeds, all_en_embeds):
    men = np.ascontiguousarray(np.asarray(men_embeds, dtype=np.float32))
    en = np.ascontiguousarray(np.asarray(all_en_embeds, dtype=np.float32))
    zm, Zm = men[:, :D], men[:, D:]
    # mention-pair column layout: col mp -> mentions (2mp, 2mp+1)
    ezm = np.exp(np.concatenate([Zm[0::2].T, Zm[1::2].T], axis=0))
    enzm = np.exp(-np.concatenate([zm[0::2].T, zm[1::2].T], axis=0))
    ezm = np.ascontiguousarray(ezm, dtype=np.float32)
    enzm = np.ascontiguousarray(enzm, dtype=np.float32)
    # -log mention volume in psum-row layout: nlmv[p, g]
    lmv = np.sum(np.log(np.logaddexp(0.0, Zm - zm)), axis=1)  # [M]
    perm = _row_perm()
    nlmv = np.empty((128, 2), dtype=np.float32)
    for g in range(2):
        nlmv[:, g] = -lmv[perm[g * 128:(g + 1) * 128]]
    zet_all = np.empty((NCORES * 64, NS), dtype=np.float32)
    nzet_all = np.empty((NCORES * 64, NS), dtype=np.float32)
    for s in range(NCORES):
        ens = en[s * NS:(s + 1) * NS]
        zet_all[s * 64:(s + 1) * 64] = ens[:, D:].T      # Ze.T
        nzet_all[s * 64:(s + 1) * 64] = -ens[:, :D].T    # -ze.T
    return {"zet": zet_all, "nzet": nzet_all,
            "ezm": np.tile(ezm, (NCORES, 1)),
            "enzm": np.tile(enzm, (NCORES, 1)),
            "nlmv": np.tile(nlmv, (NCORES, 1))}


def _make_executable(nc):
    """Jit a shard_map over 8 cores around the single bass_exec call."""
    import jax
    from jax.sharding import Mesh, PartitionSpec
    from jax.experimental.shard_map import shard_map
    from concourse import mybir
    from concourse.bass2jax import (
        install_neuronx_cc_hook, _bass_exec_p, partition_id_tensor)

    install_neuronx_cc_hook()
    partition_name = (nc.partition_id_tensor.name
                      if nc.partition_id_tensor else None)
    in_names, out_names, out_avals = [], [], []
    for alloc in nc.m.functions[0].allocations:
        if not isinstance(alloc, mybir.MemoryLocationSet):
            continue
        name = alloc.memorylocations[0].name
        if alloc.kind == "ExternalInput":
            if name != partition_name:
                in_names.append(name)
        elif alloc.kind == "ExternalOutput":
            out_names.append(name)
            out_avals.append(jax.core.ShapedArray(
                tuple(alloc.tensor_shape), mybir.dt.np(alloc.dtype)))
    all_in_names = list(in_names)
    if partition_name is not None:
        all_in_names.append(partition_name)

    def _body(*args):
        operands = list(args)
        if partition_name is not None:
            operands.append(partition_id_tensor())
        return tuple(_bass_exec_p.bind(
            *operands,
            out_avals=tuple(out_avals),
            in_names=tuple(all_in_names),
            out_names=tuple(out_names),
            lowering_input_output_aliases=(),
            sim_require_finite=True,
            sim_require_nnan=True,
            nc=nc,
        ))

    devices = jax.devices()[:NCORES]
    mesh = Mesh(np.asarray(devices), ("core",))
    fn = jax.jit(shard_map(
        _body, mesh=mesh,
        in_specs=(PartitionSpec("core"),) * len(in_names),
        out_specs=(PartitionSpec("core"),) * len(out_names),
        check_rep=False))
    return fn, in_names, mesh


def _get_state():
    if "fn" not in _cache:
        nc = _build()
        fn, in_names, mesh = _make_executable(nc)
        _cache.update(nc=nc, fn=fn, in_names=in_names, mesh=mesh,
                      perm=_row_perm(), host_in={}, dev_in={})
    return _cache


def _device_inputs(st, host_in):
    """Sharded device_put per input array, reusing cached device arrays
    for arrays whose content is unchanged since the previous call."""
    import jax
    from jax.sharding import NamedSharding, PartitionSpec
    sh = NamedSharding(st["mesh"], PartitionSpec("core"))
    dev = []
    for k in st["in_names"]:
        cached = st["host_in"].get(k)
        if cached is None or not np.array_equal(cached, host_in[k]):
            st["dev_in"][k] = jax.device_put(host_in[k], sh)
            st["host_in"][k] = host_in[k]
        dev.append(st["dev_in"][k])
    return dev


def kernel(men_embeds, all_en_embeds):
    st = _get_state()
    dev = _device_inputs(st, _prep_host(men_embeds, all_en_embeds))
    out_arrs = st["fn"](*dev)
    glob = np.asarray(out_arrs[0])                  # [8*256, 2500] bf16
    perm = st["perm"]
    out = np.empty((M, N), dtype=np.float32)
    blocks = glob.astype(np.float32).reshape(NCORES, M, NS)
    for s in range(NCORES):
        out[perm, s * NS:(s + 1) * NS] = blocks[s]
    return out


def hw_exec_time_ns(men_embeds, all_en_embeds, r_lo=17, r_hi=33, ntrials=13):
    """Per-execution on-device time, measured as the wall-time slope
    between NEFFs running the compute loop r_lo vs r_hi times internally
    (identical dispatch/transfer overhead cancels out)."""
    import time
    import jax
    st = _get_state()
    dev = _device_inputs(st, _prep_host(men_embeds, all_en_embeds))
    key = ("fns_timing", r_lo, r_hi)
    if key not in st:
        fns = {}
        for r in (r_lo, r_hi):
            fn_r, _, _ = _make_executable(_build(reps=r))
            fns[r] = fn_r
        st[key] = fns
    fns = st[key]
    for fn in fns.values():                        # warm/compile
        jax.block_until_ready(fn(*dev))
    walls = {r: [] for r in fns}
    for _ in range(ntrials):
        for r in (r_lo, r_hi):                     # adjacent in time so
            t0 = time.time()                       # dispatch conditions pair
            jax.block_until_ready(fns[r](*dev))
            walls[r].append(time.time() - t0)
    diffs = sorted(h - l for l, h in zip(walls[r_lo], walls[r_hi]))
    med_diff = diffs[len(diffs) // 2]
    return med_diff / (r_hi - r_lo) * 1e9, walls
